# revision 14
# baseline (speedup 1.0000x reference)
"""Trainium2 Bass kernel for nn_AttentionBlock (GroupNorm -> MHA -> proj + residual).

Contract: kernel(**inputs) takes the FULL unsharded inputs (as produced by
setup_inputs) and returns the FULL output [8, 512, 32, 32] float32.

Sharding: pure data-parallel over batch B=8 across the 8 NeuronCores; each core
processes one batch element end-to-end (no collectives needed).

Per-core layout / algorithm (B=1, C=512, N=H*W=1024, heads=8, head_dim=64):
  - GroupNorm(32 groups): channel-partition layout [128, 4, 1024]; per-channel
    mean/var via bn_stats/bn_aggr, group-combine + broadcast via tiny PE
    matmuls, pipelined per channel-tile (groups never cross a 128-channel tile).
  - qkv 1x1-conv as matmuls with host-pre-transposed weights (out = lhsT.T @ rhs);
    q scale (1/8) folded into wq/bq on host.
  - Attention per head in "S^T" layout: S^T[m,n] = sum_c k[c,m] q[c,n] computed
    with lhsT=k (K=64), softmax denominators come out of the AV matmul for free:
    lhsT = [v_head (64 cols) | ones (64 cols)] so PSUM rows 64:128 hold the
    denominator already broadcast across 64 partitions; exp(S) on ScalarE with
    no max subtraction (|S| <= ~8 for this distribution, fp32-safe). S tiles are
    double-buffered in PSUM and the AV matmul is software-pipelined one step
    behind exp so the PE never waits on ScalarE.
  - v-bias and proj-bias folded on host: pb_eff = proj_b + proj_w @ b_v.
  - proj matmul + residual add, output [512, 1024] fp32.
"""

import numpy as np
import ml_dtypes

import concourse.bass as bass
import concourse.tile as tile
from concourse import bacc, mybir
from concourse.bass_utils import run_bass_kernel_spmd

FP32 = mybir.dt.float32
BF16 = mybir.dt.bfloat16
AF = mybir.ActivationFunctionType
OP = mybir.AluOpType

P = 128      # SBUF partitions
C = 512      # channels
NT = 1024    # spatial tokens (32*32)
CT = C // P  # channel tiles = 4
MT = NT // P # m (key) tiles = 8
NH = 8       # heads
HD = 64      # head dim
NCORES = 8
GSZ = 16     # channels per group (512/32)

# build-time knobs (bisect/perf experiments; defaults = fastest correct config)
PIPELINE_AV = True
FAST_RECIP = True
DEBUG_ATTN = False


def _emit(tc: "tile.TileContext", io: dict):
    nc = tc.nc
    x, wq, wk, wv, pw = io["x"], io["wq"], io["wk"], io["wv"], io["pw"]
    bq, bk, pb = io["bq"], io["bk"], io["pb"]
    gg, gb = io["gg"], io["gb"]
    amat, imat = io["amat"], io["imat"]
    out = io["out"]

    import contextlib
    ctx = contextlib.ExitStack()
    with ctx:
        pers = ctx.enter_context(tc.tile_pool(name="pers", bufs=1))
        sm = ctx.enter_context(tc.tile_pool(name="small", bufs=1))

        # ---------------- input DMAs ----------------
        # order: x + small tensors first (GroupNorm's critical path), then the
        # big weights; wv/pw ride the gpsimd queue to run in parallel
        x_r = x.rearrange("(r p) n -> p r n", p=P)
        amat_sb = pers.tile([P, NH], FP32, tag="amat")
        nc.sync.dma_start(amat_sb, amat)
        imat_sb = pers.tile([NH, P], FP32, tag="imat")
        nc.sync.dma_start(imat_sb, imat)
        gg_sb = pers.tile([P, CT], FP32, tag="gg")
        nc.sync.dma_start(gg_sb, gg.rearrange("(r p) -> p r", p=P))
        gb_sb = pers.tile([P, CT], FP32, tag="gb")
        nc.sync.dma_start(gb_sb, gb.rearrange("(r p) -> p r", p=P))
        bq_sb = pers.tile([P, CT], FP32, tag="bq")
        nc.sync.dma_start(bq_sb, bq.rearrange("(r p) -> p r", p=P))
        bk_sb = pers.tile([P, CT], FP32, tag="bk")
        nc.sync.dma_start(bk_sb, bk.rearrange("(r p) -> p r", p=P))
        pb_sb = pers.tile([P, CT], FP32, tag="pb")
        nc.sync.dma_start(pb_sb, pb.rearrange("(r p) -> p r", p=P))
        x_sb = pers.tile([P, CT, NT], FP32, tag="x")
        for r in range(CT):  # split so GN tile r starts as soon as slice r lands
            nc.sync.dma_start(x_sb[:, r, :], x_r[:, r, :])
        wq_sb = pers.tile([P, CT, C], BF16, tag="wq")
        nc.scalar.dma_start(wq_sb, wq.rearrange("(k p) o -> p k o", p=P))
        wk_sb = pers.tile([P, CT, C], BF16, tag="wk")
        nc.scalar.dma_start(wk_sb, wk.rearrange("(k p) o -> p k o", p=P))
        wv_sb = pers.tile([P, CT, C], BF16, tag="wv")
        nc.gpsimd.dma_start(wv_sb, wv.rearrange("(k p) o -> p k o", p=P))
        pw_sb = pers.tile([P, CT, C], BF16, tag="pw")
        nc.gpsimd.dma_start(pw_sb, pw.rearrange("(k p) o -> p k o", p=P))
        eps_sb = pers.tile([NH, 1], FP32, tag="eps")
        nc.vector.memset(eps_sb, 1e-5)

        # v^T with interleaved ones columns: per head 128 cols = [v(64) | ones(64)]
        vT_sb = pers.tile([P, MT, NH * 128], BF16, tag="vT")
        nc.gpsimd.memset(vT_sb, 1.0)

        h_sb = pers.tile([P, CT, NT], BF16, tag="h")
        q_sb = pers.tile([P, CT, NT], BF16, tag="q")
        k_sb = pers.tile([P, CT, NT], BF16, tag="k")
        O_sb = pers.tile([P, CT, NT], BF16, tag="O")
        xpb_sb = pers.tile([P, CT, NT], FP32, tag="xpb")

        # ---------------- GroupNorm (pipelined per channel-tile) ----------------
        # groups are 16 channels wide so every group lives inside one
        # 128-channel tile: each tile's stats/normalization is independent
        with nc.named_scope("gn"), \
             tc.tile_pool(name="gnps", bufs=2, space="PSUM") as gnps, \
             tc.tile_pool(name="mrps", bufs=2, space="PSUM") as mrps:
            for r in range(CT):
                st = sm.tile([P, 2, 6], FP32, tag=f"bnstats{r}")
                nc.vector.bn_stats(st[:, 0, :], x_sb[:, r, 0:512])
                nc.vector.bn_stats(st[:, 1, :], x_sb[:, r, 512:1024])
                mv = sm.tile([P, 2], FP32, tag=f"mv{r}")
                nc.vector.bn_aggr(mv, st)
                st2 = sm.tile([P, 2], FP32, tag=f"st2{r}")
                nc.vector.tensor_copy(st2[:, 0:1], mv[:, 0:1])
                nc.vector.tensor_tensor(st2[:, 1:2], mv[:, 0:1], mv[:, 0:1], OP.mult)
                nc.vector.tensor_tensor(st2[:, 1:2], st2[:, 1:2], mv[:, 1:2], OP.add)
                # per-group (mean, m2): contract channels-in-tile with A (1/16)
                G_ps = gnps.tile([NH, 2], FP32, tag="gps", name=f"gps{r}")
                nc.tensor.matmul(G_ps, amat_sb, st2, start=True, stop=True)
                gst = sm.tile([NH, 2], FP32, tag=f"gst{r}")
                nc.vector.tensor_copy(gst, G_ps)
                var = sm.tile([NH, 1], FP32, tag=f"gvar{r}")
                nc.vector.tensor_tensor(var, gst[:, 0:1], gst[:, 0:1], OP.mult)
                nc.vector.tensor_tensor(var, gst[:, 1:2], var, OP.subtract)
                # rstd = exp(-0.5 * ln(var + eps)) (stays in the exp table sets)
                nc.scalar.activation(var, var, AF.Ln, bias=eps_sb)
                nc.scalar.activation(gst[:, 1:2], var, AF.Exp, scale=-0.5)
                # broadcast (mean, rstd) back to the tile's 128 channels
                MR_ps = mrps.tile([P, 2], FP32, tag="mrps", name=f"mrps{r}")
                nc.tensor.matmul(MR_ps, imat_sb, gst, start=True, stop=True)
                mr = sm.tile([P, 2], FP32, tag=f"mr{r}")
                nc.vector.tensor_copy(mr, MR_ps)
                a_sb = sm.tile([P, 1], FP32, tag=f"gn_a{r}")
                nc.vector.tensor_tensor(a_sb, mr[:, 1:2], gg_sb[:, r:r + 1],
                                        OP.mult)
                b_sb = sm.tile([P, 1], FP32, tag=f"gn_b{r}")
                nc.vector.tensor_tensor(b_sb, mr[:, 0:1], a_sb, OP.mult)
                nc.vector.tensor_tensor(b_sb, gb_sb[:, r:r + 1], b_sb,
                                        OP.subtract)
                nc.vector.tensor_scalar(h_sb[:, r, :], x_sb[:, r, :],
                                        a_sb, b_sb, OP.mult, OP.add)

        # ------------- qkv + attention (interleaved on PE) -------------
        # PSUM budget (4096 fp32/partition): S chunks [128,2,512] x2 bufs
        # (2048) + O pair-half [128,2,512] (1024) + background qkv/vT
        # accumulators [128,512] x2 bufs (1024). The ScalarE exp stream is the
        # attention bottleneck, so the remaining qkv matmuls are drip-fed into
        # the PE stream between attention chunks.
        from collections import deque
        with nc.named_scope("qkv_attn"), \
             tc.tile_pool(name="bgps", bufs=2, space="PSUM") as bgps, \
             tc.tile_pool(name="spool", bufs=2, space="PSUM") as spool, \
             tc.tile_pool(name="opool", bufs=1, space="PSUM") as opool, \
             tc.tile_pool(name="epool", bufs=6) as epool, \
             tc.tile_pool(name="rpool", bufs=2) as rpool:

            def qk_task(dst, w_sb, b_sb, r, half):
                ps = bgps.tile([P, 512], FP32, tag="bgps",
                               name=f"qk_{r}_{half}_{w_sb.name}")
                for kc in range(CT):
                    nc.tensor.matmul(
                        ps, w_sb[:, kc, P * r:P * r + P],
                        h_sb[:, kc, 512 * half:512 * half + 512],
                        start=(kc == 0), stop=(kc == CT - 1))
                nc.vector.tensor_scalar(dst[:, r, 512 * half:512 * half + 512],
                                        ps, b_sb[:, r:r + 1], None, OP.add)

            def vt_task(t):
                ps = bgps.tile([P, 512], FP32, tag="bgps", name=f"vt{t}")
                for kc in range(CT):
                    nc.tensor.matmul(ps, h_sb[:, kc, P * t:P * t + P],
                                     wv_sb[:, kc, :],
                                     start=(kc == 0), stop=(kc == CT - 1))
                nc.vector.tensor_copy(
                    vT_sb[:, t, :].rearrange("p (h c) -> p h c", c=128)[:, :, 0:HD],
                    ps.rearrange("p (h c) -> p h c", c=HD))

            # upfront: pair-0 dependencies (q0, k0, all of v^T)
            for half in range(2):
                qk_task(q_sb, wq_sb, bq_sb, 0, half)
            for half in range(2):
                qk_task(k_sb, wk_sb, bk_sb, 0, half)
            for t in range(5):
                vt_task(t)

            # remaining work, drip-fed during attention pairs 0..2
            bg = deque()
            for t in range(5, MT):
                bg.append((vt_task, (t,)))
            for r in range(1, CT):
                for dst, w_sb, b_sb in ((q_sb, wq_sb, bq_sb),
                                        (k_sb, wk_sb, bk_sb)):
                    for half in range(2):
                        bg.append((qk_task, (dst, w_sb, b_sb, r, half)))

            ci = 0
            for pr in range(NH // 2):
                h0 = 2 * pr
                for half in range(2):
                    hs = 512 * half
                    O_half = opool.tile([P, 2, 512], FP32, tag="oh",
                                        name=f"oh{pr}_{half}")
                    prev = None
                    for t in range(MT):
                        S_c = spool.tile([P, 2, 512], FP32, tag="sc",
                                         name=f"s{pr}_{half}_{t}")
                        for hi in range(2):
                            nc.tensor.matmul(
                                S_c[:, hi, :],
                                k_sb[HD * hi:HD * hi + HD, pr, P * t:P * t + P],
                                q_sb[HD * hi:HD * hi + HD, pr, hs:hs + 512],
                                start=True, stop=True)
                        E_c = epool.tile([P, 2, 512], BF16, tag="e",
                                         name=f"e{pr}_{half}_{t}")
                        nc.scalar.activation(E_c, S_c, AF.Exp)
                        if prev is not None:
                            tp, Ep = prev
                            for hi in range(2):
                                nc.tensor.matmul(
                                    O_half[:, hi, :],
                                    vT_sb[:, tp, 128 * (h0 + hi):128 * (h0 + hi) + 128],
                                    Ep[:, hi, :],
                                    start=(tp == 0), stop=(tp == MT - 1))
                        prev = (t, E_c)
                        ci += 1
                        if bg and (ci % 3 == 0 or len(bg) > 12):
                            fn, args = bg.popleft()
                            fn(*args)
                    tp, Ep = prev
                    for hi in range(2):
                        nc.tensor.matmul(
                            O_half[:, hi, :],
                            vT_sb[:, tp, 128 * (h0 + hi):128 * (h0 + hi) + 128],
                            Ep[:, hi, :],
                            start=(tp == 0), stop=(tp == MT - 1))
                    # epilogue: two fast PSUM->SBUF copies release the O
                    # slot; the denominators land in a dedicated base-0 packed
                    # tile because the custom-DVE recip only handles
                    # whole-tile zero-offset sources correctly
                    Ocp = rpool.tile([HD, 2, 512], FP32, tag="ocp",
                                     name=f"ocp{pr}_{half}")
                    nc.vector.tensor_copy(Ocp, O_half[0:HD, :, :])
                    Dt = rpool.tile([HD, 2, 512], FP32, tag="dt",
                                    name=f"dt{pr}_{half}")
                    nc.vector.tensor_copy(Dt, O_half[HD:128, :, :])
                    Rh = rpool.tile([HD, 2, 512], FP32, tag="rh",
                                    name=f"rh{pr}_{half}")
                    if FAST_RECIP:
                        nc.vector.reciprocal_approx_fast(Rh, Dt)
                    else:
                        nc.vector.reciprocal(Rh, Dt)
                    for hi in range(2):
                        nc.vector.tensor_tensor(
                            O_sb[HD * hi:HD * hi + HD, pr, hs:hs + 512],
                            Ocp[:, hi, :], Rh[:, hi, :], OP.mult)
                # spread the residual-precompute over the attention phase
                nc.vector.tensor_scalar(xpb_sb[:, pr, :], x_sb[:, pr, :],
                                        pb_sb[:, pr:pr + 1], None, OP.add)

        # ---------------- proj + residual ----------------
        with nc.named_scope("proj"), \
             tc.tile_pool(name="pjps", bufs=2, space="PSUM") as pjps, \
             tc.tile_pool(name="outp", bufs=2) as outp:
            out_r = out.rearrange("(r p) n -> p r n", p=P)
            for r in range(CT):
                ps = pjps.tile([P, NT], FP32, tag="pjps")
                for half in range(2):
                    for kc in range(CT):
                        nc.tensor.matmul(
                            ps[:, 512 * half:512 * half + 512],
                            pw_sb[:, kc, P * r:P * r + P],
                            O_sb[:, kc, 512 * half:512 * half + 512],
                            start=(kc == 0), stop=(kc == CT - 1))
                o_sb = outp.tile([P, NT], FP32, tag="outsb")
                nc.vector.tensor_tensor(o_sb, ps, xpb_sb[:, r, :], OP.add)
                nc.sync.dma_start(out_r[:, r, :], o_sb)


_CACHE: dict = {}


def _build():
    if "nc" in _CACHE:
        return _CACHE["nc"]
    nc = bacc.Bacc("TRN2", target_bir_lowering=False, debug=False,
                   num_devices=NCORES)
    io = {
        "x": nc.dram_tensor("x", [C, NT], FP32, kind="ExternalInput").ap(),
        "wq": nc.dram_tensor("wq", [C, C], BF16, kind="ExternalInput").ap(),
        "wk": nc.dram_tensor("wk", [C, C], BF16, kind="ExternalInput").ap(),
        "wv": nc.dram_tensor("wv", [C, C], BF16, kind="ExternalInput").ap(),
        "pw": nc.dram_tensor("pw", [C, C], BF16, kind="ExternalInput").ap(),
        "bq": nc.dram_tensor("bq", [C], FP32, kind="ExternalInput").ap(),
        "bk": nc.dram_tensor("bk", [C], FP32, kind="ExternalInput").ap(),
        "pb": nc.dram_tensor("pb", [C], FP32, kind="ExternalInput").ap(),
        "gg": nc.dram_tensor("gg", [C], FP32, kind="ExternalInput").ap(),
        "gb": nc.dram_tensor("gb", [C], FP32, kind="ExternalInput").ap(),
        "amat": nc.dram_tensor("amat", [P, NH], FP32, kind="ExternalInput").ap(),
        "imat": nc.dram_tensor("imat", [NH, P], FP32, kind="ExternalInput").ap(),
        "out": nc.dram_tensor("out", [C, NT], FP32, kind="ExternalOutput").ap(),
    }
    if DEBUG_ATTN:
        io["dbg_den"] = nc.dram_tensor("dbg_den", [NH, HD, NT], FP32,
                                       kind="ExternalOutput").ap()
        io["dbg_rh"] = nc.dram_tensor("dbg_rh", [NH, HD, NT], FP32,
                                      kind="ExternalOutput").ap()
    with tile.TileContext(nc) as tc:
        _emit(tc, io)
    nc.compile()
    _CACHE["nc"] = nc
    return nc


def _host_prep(inputs):
    x = np.ascontiguousarray(np.asarray(inputs["x"], dtype=np.float32))
    qkv_w = np.asarray(inputs["qkv_w"], dtype=np.float32)
    qkv_b = np.asarray(inputs["qkv_b"], dtype=np.float32)
    proj_w = np.asarray(inputs["proj_w"], dtype=np.float32)
    proj_b = np.asarray(inputs["proj_b"], dtype=np.float32)
    gn_scale = np.asarray(inputs["gn_scale"], dtype=np.float32)
    gn_bias = np.asarray(inputs["gn_bias"], dtype=np.float32)

    s = np.float32(1.0 / np.sqrt(HD))
    bf = ml_dtypes.bfloat16
    shared = {
        "wq": np.ascontiguousarray((qkv_w[0:C] * s).T).astype(bf),
        "wk": np.ascontiguousarray(qkv_w[C:2 * C].T).astype(bf),
        "wv": np.ascontiguousarray(qkv_w[2 * C:3 * C].T).astype(bf),
        "pw": np.ascontiguousarray(proj_w.T).astype(bf),
        "bq": (qkv_b[0:C] * s).astype(np.float32),
        "bk": qkv_b[C:2 * C].astype(np.float32),
        # v bias and proj bias folded together: proj(o + b_v) = proj(o) + W_p b_v
        "pb": (proj_b + proj_w @ qkv_b[2 * C:3 * C]).astype(np.float32),
        "gg": gn_scale,
        "gb": gn_bias,
        # amat: [128, 8], 1/16 where channel p belongs to group j of its tile
        "amat": (np.kron(np.eye(NH, dtype=np.float32),
                         np.ones((GSZ, 1), np.float32)) / GSZ),
        # imat: [8, 128], 1.0 where channel p belongs to group j of its tile
        "imat": np.ascontiguousarray(np.kron(np.eye(NH, dtype=np.float32),
                                             np.ones((1, GSZ), np.float32))),
    }
    B = x.shape[0]
    in_maps = []
    for b in range(B):
        m = dict(shared)
        m["x"] = np.ascontiguousarray(x[b].reshape(C, NT))
        in_maps.append(m)
    return in_maps


def run(inputs, trace=False):
    nc = _build()
    in_maps = _host_prep(inputs)
    res = run_bass_kernel_spmd(nc, in_maps, list(range(NCORES)), trace=trace)
    out = np.stack([res.results[i]["out"] for i in range(NCORES)], axis=0)
    return out.reshape(len(in_maps), C, 32, 32), res


def kernel(**inputs) -> np.ndarray:
    out, _ = run(inputs, trace=False)
    return out.astype(np.float32)


# revision 15
# speedup vs baseline: 1.0544x; 1.0544x over previous
"""Trainium2 Bass kernel for nn_AttentionBlock (GroupNorm -> MHA -> proj + residual).

Contract: kernel(**inputs) takes the FULL unsharded inputs (as produced by
setup_inputs) and returns the FULL output [8, 512, 32, 32] float32.

Sharding: pure data-parallel over batch B=8 across the 8 NeuronCores; each core
processes one batch element end-to-end (no collectives needed).

Per-core layout / algorithm (B=1, C=512, N=H*W=1024, heads=8, head_dim=64):
  - GroupNorm(32 groups): channel-partition layout [128, 4, 1024]; per-channel
    mean/var via bn_stats/bn_aggr, group-combine + broadcast via tiny PE
    matmuls, pipelined per channel-tile (groups never cross a 128-channel tile).
  - qkv 1x1-conv as matmuls with host-pre-transposed weights (out = lhsT.T @ rhs);
    q scale (1/8) folded into wq/bq on host.
  - Attention per head in "S^T" layout: S^T[m,n] = sum_c k[c,m] q[c,n] computed
    with lhsT=k (K=64), softmax denominators come out of the AV matmul for free:
    lhsT = [v_head (64 cols) | ones (64 cols)] so PSUM rows 64:128 hold the
    denominator already broadcast across 64 partitions; exp(S) on ScalarE with
    no max subtraction (|S| <= ~8 for this distribution, fp32-safe). S tiles are
    double-buffered in PSUM and the AV matmul is software-pipelined one step
    behind exp so the PE never waits on ScalarE.
  - v-bias and proj-bias folded on host: pb_eff = proj_b + proj_w @ b_v.
  - proj matmul + residual add, output [512, 1024] fp32.
"""

import numpy as np
import ml_dtypes

import concourse.bass as bass
import concourse.tile as tile
from concourse import bacc, mybir
from concourse.bass_utils import run_bass_kernel_spmd

FP32 = mybir.dt.float32
BF16 = mybir.dt.bfloat16
AF = mybir.ActivationFunctionType
OP = mybir.AluOpType

P = 128      # SBUF partitions
C = 512      # channels
NT = 1024    # spatial tokens (32*32)
CT = C // P  # channel tiles = 4
MT = NT // P # m (key) tiles = 8
NH = 8       # heads
HD = 64      # head dim
NCORES = 8
GSZ = 16     # channels per group (512/32)

# build-time knobs (bisect/perf experiments; defaults = fastest correct config)
PIPELINE_AV = True
FAST_RECIP = True
DEBUG_ATTN = False


def _emit(tc: "tile.TileContext", io: dict):
    nc = tc.nc
    x, wq, wk, wv, pw = io["x"], io["wq"], io["wk"], io["wv"], io["pw"]
    bq, bk, pb = io["bq"], io["bk"], io["pb"]
    gg, gb = io["gg"], io["gb"]
    amat, imat = io["amat"], io["imat"]
    out = io["out"]

    import contextlib
    ctx = contextlib.ExitStack()
    with ctx:
        pers = ctx.enter_context(tc.tile_pool(name="pers", bufs=1))
        sm = ctx.enter_context(tc.tile_pool(name="small", bufs=1))

        # ---------------- input DMAs ----------------
        # order: x + small tensors first (GroupNorm's critical path), then the
        # big weights; wv/pw ride the gpsimd queue to run in parallel
        x_r = x.rearrange("(r p) n -> p r n", p=P)
        x_sb = pers.tile([P, CT, NT], FP32, tag="x")
        # x is the critical path: two tiles per queue, nothing ahead of it
        nc.sync.dma_start(x_sb[:, 0, :], x_r[:, 0, :])
        nc.gpsimd.dma_start(x_sb[:, 1, :], x_r[:, 1, :])
        nc.sync.dma_start(x_sb[:, 2, :], x_r[:, 2, :])
        nc.gpsimd.dma_start(x_sb[:, 3, :], x_r[:, 3, :])
        amat_sb = pers.tile([P, NH], FP32, tag="amat")
        nc.scalar.dma_start(amat_sb, amat)
        imat_sb = pers.tile([NH, P], FP32, tag="imat")
        nc.scalar.dma_start(imat_sb, imat)
        gg_sb = pers.tile([P, CT], FP32, tag="gg")
        nc.scalar.dma_start(gg_sb, gg.rearrange("(r p) -> p r", p=P))
        gb_sb = pers.tile([P, CT], FP32, tag="gb")
        nc.scalar.dma_start(gb_sb, gb.rearrange("(r p) -> p r", p=P))
        bq_sb = pers.tile([P, CT], FP32, tag="bq")
        nc.scalar.dma_start(bq_sb, bq.rearrange("(r p) -> p r", p=P))
        bk_sb = pers.tile([P, CT], FP32, tag="bk")
        nc.scalar.dma_start(bk_sb, bk.rearrange("(r p) -> p r", p=P))
        pb_sb = pers.tile([P, CT], FP32, tag="pb")
        nc.scalar.dma_start(pb_sb, pb.rearrange("(r p) -> p r", p=P))
        wq_sb = pers.tile([P, CT, C], BF16, tag="wq")
        nc.scalar.dma_start(wq_sb, wq.rearrange("(k p) o -> p k o", p=P))
        wk_sb = pers.tile([P, CT, C], BF16, tag="wk")
        nc.scalar.dma_start(wk_sb, wk.rearrange("(k p) o -> p k o", p=P))
        wv_sb = pers.tile([P, CT, C], BF16, tag="wv")
        nc.sync.dma_start(wv_sb, wv.rearrange("(k p) o -> p k o", p=P))
        pw_sb = pers.tile([P, CT, C], BF16, tag="pw")
        nc.sync.dma_start(pw_sb, pw.rearrange("(k p) o -> p k o", p=P))
        eps_sb = pers.tile([NH, 1], FP32, tag="eps")
        nc.vector.memset(eps_sb, 1e-5)

        # v^T with interleaved ones columns: per head 128 cols = [v(64) | ones(64)]
        vT_sb = pers.tile([P, MT, NH * 128], BF16, tag="vT")
        nc.gpsimd.memset(vT_sb, 1.0)

        h_sb = pers.tile([P, CT, NT], BF16, tag="h")
        q_sb = pers.tile([P, CT, NT], BF16, tag="q")
        k_sb = pers.tile([P, CT, NT], BF16, tag="k")
        O_sb = pers.tile([P, CT, NT], BF16, tag="O")
        xpb_sb = pers.tile([P, CT, NT], FP32, tag="xpb")

        # ---------------- GroupNorm (pipelined per channel-tile) ----------------
        # groups are 16 channels wide so every group lives inside one
        # 128-channel tile: each tile's stats/normalization is independent
        with nc.named_scope("gn"), \
             tc.tile_pool(name="gnps", bufs=2, space="PSUM") as gnps, \
             tc.tile_pool(name="mrps", bufs=2, space="PSUM") as mrps:
            for r in range(CT):
                st = sm.tile([P, 2, 6], FP32, tag=f"bnstats{r}")
                nc.vector.bn_stats(st[:, 0, :], x_sb[:, r, 0:512])
                nc.vector.bn_stats(st[:, 1, :], x_sb[:, r, 512:1024])
                mv = sm.tile([P, 2], FP32, tag=f"mv{r}")
                nc.vector.bn_aggr(mv, st)
                st2 = sm.tile([P, 2], FP32, tag=f"st2{r}")
                nc.vector.tensor_copy(st2[:, 0:1], mv[:, 0:1])
                nc.vector.tensor_tensor(st2[:, 1:2], mv[:, 0:1], mv[:, 0:1], OP.mult)
                nc.vector.tensor_tensor(st2[:, 1:2], st2[:, 1:2], mv[:, 1:2], OP.add)
                # per-group (mean, m2): contract channels-in-tile with A (1/16)
                G_ps = gnps.tile([NH, 2], FP32, tag="gps", name=f"gps{r}")
                nc.tensor.matmul(G_ps, amat_sb, st2, start=True, stop=True)
                gst = sm.tile([NH, 2], FP32, tag=f"gst{r}")
                nc.vector.tensor_copy(gst, G_ps)
                var = sm.tile([NH, 1], FP32, tag=f"gvar{r}")
                nc.vector.tensor_tensor(var, gst[:, 0:1], gst[:, 0:1], OP.mult)
                nc.vector.tensor_tensor(var, gst[:, 1:2], var, OP.subtract)
                # rstd = exp(-0.5 * ln(var + eps)) (stays in the exp table sets)
                nc.scalar.activation(var, var, AF.Ln, bias=eps_sb)
                nc.scalar.activation(gst[:, 1:2], var, AF.Exp, scale=-0.5)
                # broadcast (mean, rstd) back to the tile's 128 channels
                MR_ps = mrps.tile([P, 2], FP32, tag="mrps", name=f"mrps{r}")
                nc.tensor.matmul(MR_ps, imat_sb, gst, start=True, stop=True)
                mr = sm.tile([P, 2], FP32, tag=f"mr{r}")
                nc.vector.tensor_copy(mr, MR_ps)
                a_sb = sm.tile([P, 1], FP32, tag=f"gn_a{r}")
                nc.vector.tensor_tensor(a_sb, mr[:, 1:2], gg_sb[:, r:r + 1],
                                        OP.mult)
                b_sb = sm.tile([P, 1], FP32, tag=f"gn_b{r}")
                nc.vector.tensor_tensor(b_sb, mr[:, 0:1], a_sb, OP.mult)
                nc.vector.tensor_tensor(b_sb, gb_sb[:, r:r + 1], b_sb,
                                        OP.subtract)
                nc.vector.tensor_scalar(h_sb[:, r, :], x_sb[:, r, :],
                                        a_sb, b_sb, OP.mult, OP.add)

        # ------------- qkv + attention (interleaved on PE) -------------
        # PSUM budget (4096 fp32/partition): S chunks [128,2,512] x2 bufs
        # (2048) + O pair-half [128,2,512] (1024) + background qkv/vT
        # accumulators [128,512] x2 bufs (1024). The ScalarE exp stream is the
        # attention bottleneck, so the remaining qkv matmuls are drip-fed into
        # the PE stream between attention chunks.
        from collections import deque
        with nc.named_scope("qkv_attn"), \
             tc.tile_pool(name="bgps", bufs=2, space="PSUM") as bgps, \
             tc.tile_pool(name="spool", bufs=2, space="PSUM") as spool, \
             tc.tile_pool(name="opool", bufs=1, space="PSUM") as opool, \
             tc.tile_pool(name="epool", bufs=6) as epool, \
             tc.tile_pool(name="rpool", bufs=2) as rpool:

            def qk_task(dst, w_sb, b_sb, r, half):
                ps = bgps.tile([P, 512], FP32, tag="bgps",
                               name=f"qk_{r}_{half}_{w_sb.name}")
                for kc in range(CT):
                    nc.tensor.matmul(
                        ps, w_sb[:, kc, P * r:P * r + P],
                        h_sb[:, kc, 512 * half:512 * half + 512],
                        start=(kc == 0), stop=(kc == CT - 1))
                nc.vector.tensor_scalar(dst[:, r, 512 * half:512 * half + 512],
                                        ps, b_sb[:, r:r + 1], None, OP.add)

            def vt_task(t):
                ps = bgps.tile([P, 512], FP32, tag="bgps", name=f"vt{t}")
                for kc in range(CT):
                    nc.tensor.matmul(ps, h_sb[:, kc, P * t:P * t + P],
                                     wv_sb[:, kc, :],
                                     start=(kc == 0), stop=(kc == CT - 1))
                nc.vector.tensor_copy(
                    vT_sb[:, t, :].rearrange("p (h c) -> p h c", c=128)[:, :, 0:HD],
                    ps.rearrange("p (h c) -> p h c", c=HD))

            # upfront: pair-0 dependencies (q0, k0, all of v^T)
            for half in range(2):
                qk_task(q_sb, wq_sb, bq_sb, 0, half)
            for half in range(2):
                qk_task(k_sb, wk_sb, bk_sb, 0, half)
            for t in range(5):
                vt_task(t)

            # remaining work, drip-fed during attention pairs 0..2
            bg = deque()
            for t in range(5, MT):
                bg.append((vt_task, (t,)))
            for r in range(1, CT):
                for dst, w_sb, b_sb in ((q_sb, wq_sb, bq_sb),
                                        (k_sb, wk_sb, bk_sb)):
                    for half in range(2):
                        bg.append((qk_task, (dst, w_sb, b_sb, r, half)))

            O_tiles = {}

            def emit_av(pr, half, t, E_c):
                h0, hs = 2 * pr, 512 * half
                if t == 0:
                    O_tiles[(pr, half)] = opool.tile(
                        [P, 2, 512], FP32, tag="oh", name=f"oh{pr}_{half}")
                O_half = O_tiles[(pr, half)]
                for hi in range(2):
                    nc.tensor.matmul(
                        O_half[:, hi, :],
                        vT_sb[:, t, 128 * (h0 + hi):128 * (h0 + hi) + 128],
                        E_c[:, hi, :],
                        start=(t == 0), stop=(t == MT - 1))

            def emit_epilogue(pr, half):
                h0, hs = 2 * pr, 512 * half
                O_half = O_tiles.pop((pr, half))
                # two fast PSUM->SBUF copies release the O slot; denominators
                # go to a dedicated base-0 packed tile because the custom-DVE
                # recip only handles whole-tile zero-offset sources correctly
                Ocp = rpool.tile([HD, 2, 512], FP32, tag="ocp",
                                 name=f"ocp{pr}_{half}")
                nc.vector.tensor_copy(Ocp, O_half[0:HD, :, :])
                Dt = rpool.tile([HD, 2, 512], FP32, tag="dt",
                                name=f"dt{pr}_{half}")
                nc.vector.tensor_copy(Dt, O_half[HD:128, :, :])
                Rh = rpool.tile([HD, 2, 512], FP32, tag="rh",
                                name=f"rh{pr}_{half}")
                if FAST_RECIP:
                    nc.vector.reciprocal_approx_fast(Rh, Dt)
                else:
                    nc.vector.reciprocal(Rh, Dt)
                for hi in range(2):
                    nc.vector.tensor_tensor(
                        O_sb[HD * hi:HD * hi + HD, pr, hs:hs + 512],
                        Ocp[:, hi, :], Rh[:, hi, :], OP.mult)

            # residual-precompute drips through the attention window too
            for r in range(CT):
                bg.append((lambda rr: nc.vector.tensor_scalar(
                    xpb_sb[:, rr, :], x_sb[:, rr, :],
                    pb_sb[:, rr:rr + 1], None, OP.add), (r,)))

            chunk_list = [(pr, half, t) for pr in range(NH // 2)
                          for half in range(2) for t in range(MT)]
            prev = None
            for ci, (pr, half, t) in enumerate(chunk_list):
                hs = 512 * half
                S_c = spool.tile([P, 2, 512], FP32, tag="sc",
                                 name=f"s{pr}_{half}_{t}")
                for hi in range(2):
                    nc.tensor.matmul(
                        S_c[:, hi, :],
                        k_sb[HD * hi:HD * hi + HD, pr, P * t:P * t + P],
                        q_sb[HD * hi:HD * hi + HD, pr, hs:hs + 512],
                        start=True, stop=True)
                E_c = epool.tile([P, 2, 512], BF16, tag="e",
                                 name=f"e{pr}_{half}_{t}")
                nc.scalar.activation(E_c, S_c, AF.Exp)
                if prev is not None:
                    emit_av(*prev)
                    if prev[2] == MT - 1:
                        emit_epilogue(prev[0], prev[1])
                prev = (pr, half, t, E_c)
                if bg and (ci % 3 == 2 or len(bg) > 12):
                    fn, args = bg.popleft()
                    fn(*args)
            emit_av(*prev)
            emit_epilogue(prev[0], prev[1])
            while bg:
                fn, args = bg.popleft()
                fn(*args)

        # ---------------- proj + residual ----------------
        with nc.named_scope("proj"), \
             tc.tile_pool(name="pjps", bufs=2, space="PSUM") as pjps, \
             tc.tile_pool(name="outp", bufs=2) as outp:
            out_r = out.rearrange("(r p) n -> p r n", p=P)
            for r in range(CT):
                ps = pjps.tile([P, NT], FP32, tag="pjps")
                for half in range(2):
                    for kc in range(CT):
                        nc.tensor.matmul(
                            ps[:, 512 * half:512 * half + 512],
                            pw_sb[:, kc, P * r:P * r + P],
                            O_sb[:, kc, 512 * half:512 * half + 512],
                            start=(kc == 0), stop=(kc == CT - 1))
                o_sb = outp.tile([P, NT], FP32, tag="outsb")
                nc.vector.tensor_tensor(o_sb, ps, xpb_sb[:, r, :], OP.add)
                nc.sync.dma_start(out_r[:, r, :], o_sb)


_CACHE: dict = {}


def _build():
    if "nc" in _CACHE:
        return _CACHE["nc"]
    nc = bacc.Bacc("TRN2", target_bir_lowering=False, debug=False,
                   num_devices=NCORES)
    io = {
        "x": nc.dram_tensor("x", [C, NT], FP32, kind="ExternalInput").ap(),
        "wq": nc.dram_tensor("wq", [C, C], BF16, kind="ExternalInput").ap(),
        "wk": nc.dram_tensor("wk", [C, C], BF16, kind="ExternalInput").ap(),
        "wv": nc.dram_tensor("wv", [C, C], BF16, kind="ExternalInput").ap(),
        "pw": nc.dram_tensor("pw", [C, C], BF16, kind="ExternalInput").ap(),
        "bq": nc.dram_tensor("bq", [C], FP32, kind="ExternalInput").ap(),
        "bk": nc.dram_tensor("bk", [C], FP32, kind="ExternalInput").ap(),
        "pb": nc.dram_tensor("pb", [C], FP32, kind="ExternalInput").ap(),
        "gg": nc.dram_tensor("gg", [C], FP32, kind="ExternalInput").ap(),
        "gb": nc.dram_tensor("gb", [C], FP32, kind="ExternalInput").ap(),
        "amat": nc.dram_tensor("amat", [P, NH], FP32, kind="ExternalInput").ap(),
        "imat": nc.dram_tensor("imat", [NH, P], FP32, kind="ExternalInput").ap(),
        "out": nc.dram_tensor("out", [C, NT], FP32, kind="ExternalOutput").ap(),
    }
    if DEBUG_ATTN:
        io["dbg_den"] = nc.dram_tensor("dbg_den", [NH, HD, NT], FP32,
                                       kind="ExternalOutput").ap()
        io["dbg_rh"] = nc.dram_tensor("dbg_rh", [NH, HD, NT], FP32,
                                      kind="ExternalOutput").ap()
    with tile.TileContext(nc) as tc:
        _emit(tc, io)
    nc.compile()
    _CACHE["nc"] = nc
    return nc


def _host_prep(inputs):
    x = np.ascontiguousarray(np.asarray(inputs["x"], dtype=np.float32))
    qkv_w = np.asarray(inputs["qkv_w"], dtype=np.float32)
    qkv_b = np.asarray(inputs["qkv_b"], dtype=np.float32)
    proj_w = np.asarray(inputs["proj_w"], dtype=np.float32)
    proj_b = np.asarray(inputs["proj_b"], dtype=np.float32)
    gn_scale = np.asarray(inputs["gn_scale"], dtype=np.float32)
    gn_bias = np.asarray(inputs["gn_bias"], dtype=np.float32)

    s = np.float32(1.0 / np.sqrt(HD))
    bf = ml_dtypes.bfloat16
    shared = {
        "wq": np.ascontiguousarray((qkv_w[0:C] * s).T).astype(bf),
        "wk": np.ascontiguousarray(qkv_w[C:2 * C].T).astype(bf),
        "wv": np.ascontiguousarray(qkv_w[2 * C:3 * C].T).astype(bf),
        "pw": np.ascontiguousarray(proj_w.T).astype(bf),
        "bq": (qkv_b[0:C] * s).astype(np.float32),
        "bk": qkv_b[C:2 * C].astype(np.float32),
        # v bias and proj bias folded together: proj(o + b_v) = proj(o) + W_p b_v
        "pb": (proj_b + proj_w @ qkv_b[2 * C:3 * C]).astype(np.float32),
        "gg": gn_scale,
        "gb": gn_bias,
        # amat: [128, 8], 1/16 where channel p belongs to group j of its tile
        "amat": (np.kron(np.eye(NH, dtype=np.float32),
                         np.ones((GSZ, 1), np.float32)) / GSZ),
        # imat: [8, 128], 1.0 where channel p belongs to group j of its tile
        "imat": np.ascontiguousarray(np.kron(np.eye(NH, dtype=np.float32),
                                             np.ones((1, GSZ), np.float32))),
    }
    B = x.shape[0]
    in_maps = []
    for b in range(B):
        m = dict(shared)
        m["x"] = np.ascontiguousarray(x[b].reshape(C, NT))
        in_maps.append(m)
    return in_maps


def run(inputs, trace=False):
    nc = _build()
    in_maps = _host_prep(inputs)
    res = run_bass_kernel_spmd(nc, in_maps, list(range(NCORES)), trace=trace)
    out = np.stack([res.results[i]["out"] for i in range(NCORES)], axis=0)
    return out.reshape(len(in_maps), C, 32, 32), res


def kernel(**inputs) -> np.ndarray:
    out, _ = run(inputs, trace=False)
    return out.astype(np.float32)


# revision 16
# speedup vs baseline: 1.1037x; 1.0467x over previous
"""Trainium2 Bass kernel for nn_AttentionBlock (GroupNorm -> MHA -> proj + residual).

Contract: kernel(**inputs) takes the FULL unsharded inputs (as produced by
setup_inputs) and returns the FULL output [8, 512, 32, 32] float32.

Sharding: pure data-parallel over batch B=8 across the 8 NeuronCores; each core
processes one batch element end-to-end (no collectives needed).

Per-core layout / algorithm (B=1, C=512, N=H*W=1024, heads=8, head_dim=64):
  - GroupNorm(32 groups): channel-partition layout [128, 4, 1024]; per-channel
    mean/var via bn_stats/bn_aggr, group-combine + broadcast via tiny PE
    matmuls, pipelined per channel-tile (groups never cross a 128-channel tile).
  - qkv 1x1-conv as matmuls with host-pre-transposed weights (out = lhsT.T @ rhs);
    q scale (1/8) folded into wq/bq on host.
  - Attention per head in "S^T" layout: S^T[m,n] = sum_c k[c,m] q[c,n] computed
    with lhsT=k (K=64), softmax denominators come out of the AV matmul for free:
    lhsT = [v_head (64 cols) | ones (64 cols)] so PSUM rows 64:128 hold the
    denominator already broadcast across 64 partitions; exp(S) on ScalarE with
    no max subtraction (|S| <= ~8 for this distribution, fp32-safe). S tiles are
    double-buffered in PSUM and the AV matmul is software-pipelined one step
    behind exp so the PE never waits on ScalarE.
  - v-bias and proj-bias folded on host: pb_eff = proj_b + proj_w @ b_v.
  - proj matmul + residual add, output [512, 1024] fp32.
"""

import numpy as np
import ml_dtypes

import concourse.bass as bass
import concourse.tile as tile
from concourse import bacc, mybir
from concourse.bass_utils import run_bass_kernel_spmd

FP32 = mybir.dt.float32
BF16 = mybir.dt.bfloat16
AF = mybir.ActivationFunctionType
OP = mybir.AluOpType

P = 128      # SBUF partitions
C = 512      # channels
NT = 1024    # spatial tokens (32*32)
CT = C // P  # channel tiles = 4
MT = NT // P # m (key) tiles = 8
NH = 8       # heads
HD = 64      # head dim
NCORES = 8
GSZ = 16     # channels per group (512/32)

# build-time knobs (bisect/perf experiments; defaults = fastest correct config)
PIPELINE_AV = True
FAST_RECIP = True
DEBUG_ATTN = False


def _emit(tc: "tile.TileContext", io: dict):
    nc = tc.nc
    x, wq, wk, wv, pw = io["x"], io["wq"], io["wk"], io["wv"], io["pw"]
    bq, bk, pb = io["bq"], io["bk"], io["pb"]
    gg, gb = io["gg"], io["gb"]
    amat, imat = io["amat"], io["imat"]
    out = io["out"]

    import contextlib
    ctx = contextlib.ExitStack()
    with ctx:
        pers = ctx.enter_context(tc.tile_pool(name="pers", bufs=1))
        sm = ctx.enter_context(tc.tile_pool(name="small", bufs=1))

        # ---------------- input DMAs ----------------
        # order: x + small tensors first (GroupNorm's critical path), then the
        # big weights; wv/pw ride the gpsimd queue to run in parallel
        x_r = x.rearrange("(r p) n -> p r n", p=P)
        x_sb = pers.tile([P, CT, NT], FP32, tag="x")
        # x is the critical path: two tiles per queue, nothing ahead of it
        nc.sync.dma_start(x_sb[:, 0, :], x_r[:, 0, :])
        nc.gpsimd.dma_start(x_sb[:, 1, :], x_r[:, 1, :])
        nc.sync.dma_start(x_sb[:, 2, :], x_r[:, 2, :])
        nc.gpsimd.dma_start(x_sb[:, 3, :], x_r[:, 3, :])
        amat_sb = pers.tile([P, NH], FP32, tag="amat")
        nc.scalar.dma_start(amat_sb, amat)
        imat_sb = pers.tile([NH, P], FP32, tag="imat")
        nc.scalar.dma_start(imat_sb, imat)
        gg_sb = pers.tile([P, CT], FP32, tag="gg")
        nc.scalar.dma_start(gg_sb, gg.rearrange("(r p) -> p r", p=P))
        gb_sb = pers.tile([P, CT], FP32, tag="gb")
        nc.scalar.dma_start(gb_sb, gb.rearrange("(r p) -> p r", p=P))
        bq_sb = pers.tile([P, CT], FP32, tag="bq")
        nc.scalar.dma_start(bq_sb, bq.rearrange("(r p) -> p r", p=P))
        bk_sb = pers.tile([P, CT], FP32, tag="bk")
        nc.scalar.dma_start(bk_sb, bk.rearrange("(r p) -> p r", p=P))
        pb_sb = pers.tile([P, CT], FP32, tag="pb")
        nc.scalar.dma_start(pb_sb, pb.rearrange("(r p) -> p r", p=P))
        wq_sb = pers.tile([P, CT, C], BF16, tag="wq")
        nc.scalar.dma_start(wq_sb, wq.rearrange("(k p) o -> p k o", p=P))
        wk_sb = pers.tile([P, CT, C], BF16, tag="wk")
        nc.scalar.dma_start(wk_sb, wk.rearrange("(k p) o -> p k o", p=P))
        wv_sb = pers.tile([P, CT, C], BF16, tag="wv")
        nc.sync.dma_start(wv_sb, wv.rearrange("(k p) o -> p k o", p=P))
        pw_sb = pers.tile([P, CT, C], BF16, tag="pw")
        nc.sync.dma_start(pw_sb, pw.rearrange("(k p) o -> p k o", p=P))
        eps_sb = pers.tile([NH, 1], FP32, tag="eps")
        nc.vector.memset(eps_sb, 1e-5)

        # v^T with interleaved ones columns: per head 128 cols = [v(64) | ones(64)]
        vT_sb = pers.tile([P, MT, NH * 128], BF16, tag="vT")
        nc.gpsimd.memset(vT_sb, 1.0)

        h_sb = pers.tile([P, CT, NT], BF16, tag="h")
        q_sb = pers.tile([P, CT, NT], BF16, tag="q")
        k_sb = pers.tile([P, CT, NT], BF16, tag="k")
        O_sb = pers.tile([P, CT, NT], BF16, tag="O")
        xpb_sb = pers.tile([P, CT, NT], FP32, tag="xpb")

        # ---------------- GroupNorm (pipelined per channel-tile) ----------------
        # groups are 16 channels wide so every group lives inside one
        # 128-channel tile: each tile's stats/normalization is independent
        with nc.named_scope("gn"), \
             tc.tile_pool(name="gnps", bufs=2, space="PSUM") as gnps, \
             tc.tile_pool(name="mrps", bufs=2, space="PSUM") as mrps:
            for r in range(CT):
                st = sm.tile([P, 2, 6], FP32, tag=f"bnstats{r}")
                nc.vector.bn_stats(st[:, 0, :], x_sb[:, r, 0:512])
                nc.vector.bn_stats(st[:, 1, :], x_sb[:, r, 512:1024])
                mv = sm.tile([P, 2], FP32, tag=f"mv{r}")
                nc.vector.bn_aggr(mv, st)
                st2 = sm.tile([P, 2], FP32, tag=f"st2{r}")
                nc.vector.tensor_copy(st2[:, 0:1], mv[:, 0:1])
                nc.vector.tensor_tensor(st2[:, 1:2], mv[:, 0:1], mv[:, 0:1], OP.mult)
                nc.vector.tensor_tensor(st2[:, 1:2], st2[:, 1:2], mv[:, 1:2], OP.add)
                # per-group (mean, m2): contract channels-in-tile with A (1/16)
                G_ps = gnps.tile([NH, 2], FP32, tag="gps", name=f"gps{r}")
                nc.tensor.matmul(G_ps, amat_sb, st2, start=True, stop=True)
                gst = sm.tile([NH, 2], FP32, tag=f"gst{r}")
                nc.vector.tensor_copy(gst, G_ps)
                var = sm.tile([NH, 1], FP32, tag=f"gvar{r}")
                nc.vector.tensor_tensor(var, gst[:, 0:1], gst[:, 0:1], OP.mult)
                nc.vector.tensor_tensor(var, gst[:, 1:2], var, OP.subtract)
                # rstd = exp(-0.5 * ln(var + eps)) (stays in the exp table sets)
                nc.scalar.activation(var, var, AF.Ln, bias=eps_sb)
                nc.scalar.activation(gst[:, 1:2], var, AF.Exp, scale=-0.5)
                # broadcast (mean, rstd) back to the tile's 128 channels
                MR_ps = mrps.tile([P, 2], FP32, tag="mrps", name=f"mrps{r}")
                nc.tensor.matmul(MR_ps, imat_sb, gst, start=True, stop=True)
                mr = sm.tile([P, 2], FP32, tag=f"mr{r}")
                nc.vector.tensor_copy(mr, MR_ps)
                a_sb = sm.tile([P, 1], FP32, tag=f"gn_a{r}")
                nc.vector.tensor_tensor(a_sb, mr[:, 1:2], gg_sb[:, r:r + 1],
                                        OP.mult)
                b_sb = sm.tile([P, 1], FP32, tag=f"gn_b{r}")
                nc.vector.tensor_tensor(b_sb, mr[:, 0:1], a_sb, OP.mult)
                nc.vector.tensor_tensor(b_sb, gb_sb[:, r:r + 1], b_sb,
                                        OP.subtract)
                nc.vector.tensor_scalar(h_sb[:, r, :], x_sb[:, r, :],
                                        a_sb, b_sb, OP.mult, OP.add)

        # ------------- qkv + attention (interleaved on PE) -------------
        # PSUM budget (4096 fp32/partition): S chunks [128,2,512] x2 bufs
        # (2048) + O pair-half [128,2,512] (1024) + background qkv/vT
        # accumulators [128,512] x2 bufs (1024). The ScalarE exp stream is the
        # attention bottleneck, so the remaining qkv matmuls are drip-fed into
        # the PE stream between attention chunks.
        from collections import deque
        with nc.named_scope("qkv_attn"), \
             tc.tile_pool(name="bgps", bufs=2, space="PSUM") as bgps, \
             tc.tile_pool(name="spool", bufs=2, space="PSUM") as spool, \
             tc.tile_pool(name="opool", bufs=1, space="PSUM") as opool, \
             tc.tile_pool(name="epool", bufs=6) as epool, \
             tc.tile_pool(name="rpool", bufs=2) as rpool:

            def qk_task(dst, w_sb, b_sb, r, half):
                ps = bgps.tile([P, 512], FP32, tag="bgps",
                               name=f"qk_{r}_{half}_{w_sb.name}")
                for kc in range(CT):
                    nc.tensor.matmul(
                        ps, w_sb[:, kc, P * r:P * r + P],
                        h_sb[:, kc, 512 * half:512 * half + 512],
                        start=(kc == 0), stop=(kc == CT - 1))
                nc.vector.tensor_scalar(dst[:, r, 512 * half:512 * half + 512],
                                        ps, b_sb[:, r:r + 1], None, OP.add)

            def vt_task(t):
                ps = bgps.tile([P, 512], FP32, tag="bgps", name=f"vt{t}")
                for kc in range(CT):
                    nc.tensor.matmul(ps, h_sb[:, kc, P * t:P * t + P],
                                     wv_sb[:, kc, :],
                                     start=(kc == 0), stop=(kc == CT - 1))
                nc.vector.tensor_copy(
                    vT_sb[:, t, :].rearrange("p (h c) -> p h c", c=128)[:, :, 0:HD],
                    ps.rearrange("p (h c) -> p h c", c=HD))

            # upfront: only what attention chunk 0 needs (q0/k0 first halves)
            qk_task(q_sb, wq_sb, bq_sb, 0, 0)
            qk_task(k_sb, wk_sb, bk_sb, 0, 0)

            # everything else drips into the PE stream between attention
            # chunks, scheduled against each consumer's first-use deadline
            def xpb_task(rr):
                nc.vector.tensor_scalar(xpb_sb[:, rr, :], x_sb[:, rr, :],
                                        pb_sb[:, rr:rr + 1], None, OP.add)

            drip = {
                0: [(vt_task, (0,))], 1: [(vt_task, (1,))],
                2: [(vt_task, (2,))],
                3: [(vt_task, (3,)), (qk_task, (k_sb, wk_sb, bk_sb, 0, 1))],
                4: [(vt_task, (4,))], 5: [(vt_task, (5,))],
                6: [(vt_task, (6,))],
                7: [(vt_task, (7,)), (qk_task, (q_sb, wq_sb, bq_sb, 0, 1))],
                9: [(qk_task, (q_sb, wq_sb, bq_sb, 1, 0))],
                11: [(qk_task, (k_sb, wk_sb, bk_sb, 1, 0))],
                14: [(qk_task, (k_sb, wk_sb, bk_sb, 1, 1))],
                17: [(qk_task, (q_sb, wq_sb, bq_sb, 1, 1))],
                20: [(qk_task, (q_sb, wq_sb, bq_sb, 2, 0))],
                23: [(qk_task, (k_sb, wk_sb, bk_sb, 2, 0))],
                26: [(qk_task, (k_sb, wk_sb, bk_sb, 2, 1))],
                29: [(qk_task, (q_sb, wq_sb, bq_sb, 2, 1))],
                32: [(qk_task, (q_sb, wq_sb, bq_sb, 3, 0))],
                35: [(qk_task, (k_sb, wk_sb, bk_sb, 3, 0))],
                38: [(qk_task, (k_sb, wk_sb, bk_sb, 3, 1))],
                41: [(qk_task, (q_sb, wq_sb, bq_sb, 3, 1))],
                44: [(xpb_task, (0,))], 47: [(xpb_task, (1,))],
                50: [(xpb_task, (2,))], 53: [(xpb_task, (3,))],
            }

            O_tiles = {}

            def emit_av(pr, half, t, E_c):
                h0, hs = 2 * pr, 512 * half
                if t == 0:
                    O_tiles[(pr, half)] = opool.tile(
                        [P, 2, 512], FP32, tag="oh", name=f"oh{pr}_{half}")
                O_half = O_tiles[(pr, half)]
                for hi in range(2):
                    nc.tensor.matmul(
                        O_half[:, hi, :],
                        vT_sb[:, t, 128 * (h0 + hi):128 * (h0 + hi) + 128],
                        E_c[:, 512 * hi:512 * hi + 512],
                        start=(t == 0), stop=(t == MT - 1))

            def emit_epilogue(pr, half):
                h0, hs = 2 * pr, 512 * half
                O_half = O_tiles.pop((pr, half))
                # two fast PSUM->SBUF copies release the O slot; denominators
                # go to a dedicated base-0 packed tile because the custom-DVE
                # recip only handles whole-tile zero-offset sources correctly
                Ocp = rpool.tile([HD, 2, 512], FP32, tag="ocp",
                                 name=f"ocp{pr}_{half}")
                nc.vector.tensor_copy(Ocp, O_half[0:HD, :, :])
                Dt = rpool.tile([HD, 2, 512], FP32, tag="dt",
                                name=f"dt{pr}_{half}")
                nc.vector.tensor_copy(Dt, O_half[HD:128, :, :])
                Rh = rpool.tile([HD, 2, 512], FP32, tag="rh",
                                name=f"rh{pr}_{half}")
                if FAST_RECIP:
                    nc.vector.reciprocal_approx_fast(Rh, Dt)
                else:
                    nc.vector.reciprocal(Rh, Dt)
                for hi in range(2):
                    nc.vector.tensor_tensor(
                        O_sb[HD * hi:HD * hi + HD, pr, hs:hs + 512],
                        Ocp[:, hi, :], Rh[:, hi, :], OP.mult)

            chunk_list = [(pr, half, t) for pr in range(NH // 2)
                          for half in range(2) for t in range(MT)]
            prev = None
            for ci, (pr, half, t) in enumerate(chunk_list):
                hs = 512 * half
                S_c = spool.tile([P, 1024], FP32, tag="sc",
                                 name=f"s{pr}_{half}_{t}")
                for hi in range(2):
                    nc.tensor.matmul(
                        S_c[:, 512 * hi:512 * hi + 512],
                        k_sb[HD * hi:HD * hi + HD, pr, P * t:P * t + P],
                        q_sb[HD * hi:HD * hi + HD, pr, hs:hs + 512],
                        start=True, stop=True)
                E_c = epool.tile([P, 1024], BF16, tag="e",
                                 name=f"e{pr}_{half}_{t}")
                nc.scalar.activation(E_c, S_c, AF.Exp)
                if prev is not None:
                    emit_av(*prev)
                    if prev[2] == MT - 1:
                        emit_epilogue(prev[0], prev[1])
                prev = (pr, half, t, E_c)
                for fn, args in drip.pop(ci, ()):
                    fn(*args)
            emit_av(*prev)
            emit_epilogue(prev[0], prev[1])
            assert not drip

        # ---------------- proj + residual ----------------
        with nc.named_scope("proj"), \
             tc.tile_pool(name="pjps", bufs=2, space="PSUM") as pjps, \
             tc.tile_pool(name="outp", bufs=2) as outp:
            out_r = out.rearrange("(r p) n -> p r n", p=P)
            for r in range(CT):
                ps = pjps.tile([P, NT], FP32, tag="pjps")
                for half in range(2):
                    for kc in range(CT):
                        nc.tensor.matmul(
                            ps[:, 512 * half:512 * half + 512],
                            pw_sb[:, kc, P * r:P * r + P],
                            O_sb[:, kc, 512 * half:512 * half + 512],
                            start=(kc == 0), stop=(kc == CT - 1))
                o_sb = outp.tile([P, NT], FP32, tag="outsb")
                nc.vector.tensor_tensor(o_sb, ps, xpb_sb[:, r, :], OP.add)
                eng = nc.sync if r % 2 == 0 else nc.gpsimd
                eng.dma_start(out_r[:, r, :], o_sb)


_CACHE: dict = {}


def _build():
    if "nc" in _CACHE:
        return _CACHE["nc"]
    nc = bacc.Bacc("TRN2", target_bir_lowering=False, debug=False,
                   num_devices=NCORES)
    io = {
        "x": nc.dram_tensor("x", [C, NT], FP32, kind="ExternalInput").ap(),
        "wq": nc.dram_tensor("wq", [C, C], BF16, kind="ExternalInput").ap(),
        "wk": nc.dram_tensor("wk", [C, C], BF16, kind="ExternalInput").ap(),
        "wv": nc.dram_tensor("wv", [C, C], BF16, kind="ExternalInput").ap(),
        "pw": nc.dram_tensor("pw", [C, C], BF16, kind="ExternalInput").ap(),
        "bq": nc.dram_tensor("bq", [C], FP32, kind="ExternalInput").ap(),
        "bk": nc.dram_tensor("bk", [C], FP32, kind="ExternalInput").ap(),
        "pb": nc.dram_tensor("pb", [C], FP32, kind="ExternalInput").ap(),
        "gg": nc.dram_tensor("gg", [C], FP32, kind="ExternalInput").ap(),
        "gb": nc.dram_tensor("gb", [C], FP32, kind="ExternalInput").ap(),
        "amat": nc.dram_tensor("amat", [P, NH], FP32, kind="ExternalInput").ap(),
        "imat": nc.dram_tensor("imat", [NH, P], FP32, kind="ExternalInput").ap(),
        "out": nc.dram_tensor("out", [C, NT], FP32, kind="ExternalOutput").ap(),
    }
    if DEBUG_ATTN:
        io["dbg_den"] = nc.dram_tensor("dbg_den", [NH, HD, NT], FP32,
                                       kind="ExternalOutput").ap()
        io["dbg_rh"] = nc.dram_tensor("dbg_rh", [NH, HD, NT], FP32,
                                      kind="ExternalOutput").ap()
    with tile.TileContext(nc) as tc:
        _emit(tc, io)
    nc.compile()
    _CACHE["nc"] = nc
    return nc


def _host_prep(inputs):
    x = np.ascontiguousarray(np.asarray(inputs["x"], dtype=np.float32))
    qkv_w = np.asarray(inputs["qkv_w"], dtype=np.float32)
    qkv_b = np.asarray(inputs["qkv_b"], dtype=np.float32)
    proj_w = np.asarray(inputs["proj_w"], dtype=np.float32)
    proj_b = np.asarray(inputs["proj_b"], dtype=np.float32)
    gn_scale = np.asarray(inputs["gn_scale"], dtype=np.float32)
    gn_bias = np.asarray(inputs["gn_bias"], dtype=np.float32)

    s = np.float32(1.0 / np.sqrt(HD))
    bf = ml_dtypes.bfloat16
    shared = {
        "wq": np.ascontiguousarray((qkv_w[0:C] * s).T).astype(bf),
        "wk": np.ascontiguousarray(qkv_w[C:2 * C].T).astype(bf),
        "wv": np.ascontiguousarray(qkv_w[2 * C:3 * C].T).astype(bf),
        "pw": np.ascontiguousarray(proj_w.T).astype(bf),
        "bq": (qkv_b[0:C] * s).astype(np.float32),
        "bk": qkv_b[C:2 * C].astype(np.float32),
        # v bias and proj bias folded together: proj(o + b_v) = proj(o) + W_p b_v
        "pb": (proj_b + proj_w @ qkv_b[2 * C:3 * C]).astype(np.float32),
        "gg": gn_scale,
        "gb": gn_bias,
        # amat: [128, 8], 1/16 where channel p belongs to group j of its tile
        "amat": (np.kron(np.eye(NH, dtype=np.float32),
                         np.ones((GSZ, 1), np.float32)) / GSZ),
        # imat: [8, 128], 1.0 where channel p belongs to group j of its tile
        "imat": np.ascontiguousarray(np.kron(np.eye(NH, dtype=np.float32),
                                             np.ones((1, GSZ), np.float32))),
    }
    B = x.shape[0]
    in_maps = []
    for b in range(B):
        m = dict(shared)
        m["x"] = np.ascontiguousarray(x[b].reshape(C, NT))
        in_maps.append(m)
    return in_maps


def run(inputs, trace=False):
    nc = _build()
    in_maps = _host_prep(inputs)
    res = run_bass_kernel_spmd(nc, in_maps, list(range(NCORES)), trace=trace)
    out = np.stack([res.results[i]["out"] for i in range(NCORES)], axis=0)
    return out.reshape(len(in_maps), C, 32, 32), res


def kernel(**inputs) -> np.ndarray:
    out, _ = run(inputs, trace=False)
    return out.astype(np.float32)
